# revision 30
# baseline (speedup 1.0000x reference)
"""Multi-head attention (LoRA QKV + ALiBi + causal softmax + output proj) on 8 TRN2 cores.

Sharding: core = (batch b in 0..3, head-group hg in 0..1); each core handles one batch
element and 8 of the 16 heads.  LoRA is folded into effective weights on the host
(W_eff = W + 2*A@B, exact algebra).  Each core computes a partial projection output
(its 512 attention dims x full Wp rows); the host sums the two partials per batch.

On-core math:
  qT[d,t] = sum_e wqT[e,d] * xT[e,t]          (wqT pre-scaled by 1/sqrt(dh); f32r)
  kT[d,t], v[t,d] similar; qT/kT stored bf16
  sT[j,i] = sum_d kT[d,j] qT[d,i]             (two heads row-tiled concurrently on the
                                               PE: 64-contraction pairs at partitions
                                               0:64 / 64:128 share the array)
  p[j,i]  = exp(sT[j,i])  (bf16)              (analytic softmax max slope*i + C folded:
                                               the -slope*j - C factor is baked into v')
  causal: p[j,i] = 0 where j > i              (gpsimd affine_select on diagonal tiles)
  pv[d,i] = sum_j v'[j,d] p[j,i]              (v' bf16 has a ones column -> row d=64 is
                                               the softmax denominator)
  outT[d,i] = pv[d,i] / pv[64,i]              (reciprocal_approx_fast on DVE + a
                                               partition-replicating SBUF->SBUF DMA for
                                               the broadcast -- no PE, no PSUM)
  out[t,e] = sum_d outT[d,t] * wpT[d,e]       (bf16; partial, host adds the other group)

Schedule: chunk-major (c outer, head-pair inner).  The ACT engine's exp is the
attention-phase rate limiter (~1.1us per j-tile vs ~0.65us of PE work), so the PE
stream is kept dense by (1) lagging each PV pair two j-tiles behind its S pair and
(2) pumping single-matmul "filler" steps (V tiles, QK projections for later chunks,
output-projection groups) between attention matmuls at j-tile granularity.  A dense
PE stream also keeps the HAM clock gate at 8/8 (2.4 GHz); the baseline's chunk-level
interleave left ~0.4us PE gaps per j-tile and oscillated at 4/8 half the time.
"""

import math
from collections import deque
from contextlib import ExitStack

import numpy as np

import concourse.bacc as bacc
import concourse.mybir as mybir
import concourse.tile as tile
from concourse.bass_utils import run_bass_kernel_spmd

T, E, DH, H = 2048, 1024, 64, 16
HL = 8              # heads per core
NKT = 8             # contraction tiles of 128 over E
NTT = 16            # token tiles of 128 over T
CB = 12.0           # safety constant in the analytic softmax max
LAG = 3             # j-tiles a PV pair trails its S pair
PUMP = 3            # filler steps pumped per j-tile

_NC_CACHE = None


def _build_nc():
    f32 = mybir.dt.float32
    f32r = mybir.dt.float32r
    bf16 = mybir.dt.bfloat16
    Exp = mybir.ActivationFunctionType.Exp

    nc = bacc.Bacc(trn_type="TRN2", target_bir_lowering=False, debug=False)
    xT_d = nc.declare_dram_parameter("xT", [E, T], bf16, isOutput=False)
    wqT_d = nc.declare_dram_parameter("wqT", [E, 512], bf16, isOutput=False)
    wkT_d = nc.declare_dram_parameter("wkT", [E, 512], bf16, isOutput=False)
    wvT_d = nc.declare_dram_parameter("wvT", [E, 512], bf16, isOutput=False)
    wpT_d = nc.declare_dram_parameter("wpT", [512, E], bf16, isOutput=False)
    eb_d = nc.declare_dram_parameter("ebias", [128, 128], f32, isOutput=False)
    ones_d = nc.declare_dram_parameter("onesd", [128, 128], f32, isOutput=False)
    out_d = nc.declare_dram_parameter("out", [T, E], f32, isOutput=True)

    with ExitStack() as st:
        tc = st.enter_context(tile.TileContext(nc))
        ps = st.enter_context(tc.tile_pool(name="ps", bufs=1, space="PSUM"))
        # psum tags: acc(2, shared with norm bp) + s(4) + pv(2) = 8 banks
        sb_r = st.enter_context(tc.tile_pool(name="sbr", bufs=1, side="right"))
        sb_l = st.enter_context(tc.tile_pool(name="sbl", bufs=1, side="left"))

        # ---------- persistent SBUF tiles ----------
        xts = [sb_l.tile([128, T], bf16, tag=f"xt{k}", bufs=1, name=f"xt{k}")
               for k in range(NKT)]
        qts = [sb_l.tile([128, T], bf16, tag=f"qt{hp}", bufs=1, name=f"qt{hp}")
               for hp in range(4)]
        kts = [sb_l.tile([128, T], bf16, tag=f"kt{hp}", bufs=1, name=f"kt{hp}")
               for hp in range(4)]
        oTs = [sb_r.tile([128, T], bf16, tag=f"ot{hp}", bufs=1, name=f"ot{hp}")
               for hp in range(4)]
        vts = [sb_r.tile([128, HL * 65], bf16, tag=f"v{tt}", bufs=1, name=f"v{tt}")
               for tt in range(NTT)]
        gv_sb = sb_r.tile([128, 128], f32, tag="gv", bufs=1)
        ones_t = sb_r.tile([128, 64], f32r, tag="ones", bufs=1)
        nc.gpsimd.dma_start(out=ones_t[:], in_=ones_d[:, 0:64].bitcast(f32r))

        # ---------- DMA plumbing ----------
        # sync queue: xT chunks in order + output writes (late).  gpsimd queue:
        # gv/ones + weights in first-needed order.  ACT queue: tiny norm moves
        # (low latency; its triggers are cheap on the Scalar engine).
        nc.gpsimd.dma_start(out=gv_sb[:], in_=eb_d[:])

        def emit_xchunk_dma(ck):
            for k in range(NKT):
                nc.sync.dma_start(
                    out=xts[k][:, ck * 512:(ck + 1) * 512],
                    in_=xT_d[k * 128:(k + 1) * 128,
                             ck * 512:(ck + 1) * 512])

        for ck in range(4):
            emit_xchunk_dma(ck)
        wqk = []

        def emit_wqk_dma(hp):
            tiles = {}
            for which, wd in (("q", wqT_d), ("k", wkT_d)):
                wt = sb_l.tile([128, 1024], bf16, tag="wqk", bufs=8,
                               name=f"w{which}{hp}")
                src = wd[:, hp * 128:(hp + 1) * 128]
                src = src.rearrange("(k p) m -> p k m", p=128)
                nc.gpsimd.dma_start(out=wt.rearrange("p (k m) -> p k m", k=NKT), in_=src)
                tiles[which] = wt
            wqk.append(tiles)

        emit_wqk_dma(0)
        wvs = []
        for k in range(NKT):
            t = sb_l.tile([128, 512], bf16, tag="wst", bufs=8, name=f"wv{k}")
            nc.gpsimd.dma_start(out=t[:], in_=wvT_d[k * 128:(k + 1) * 128, :])
            wvs.append(t)
        for hp in range(1, 4):
            emit_wqk_dma(hp)
        wps = []
        for i in range(8):  # i = hp*2 + ec
            hp, ec = i // 2, i % 2
            t = sb_l.tile([128, 512], bf16, tag="wpt", bufs=8, name=f"wp{i}")
            nc.gpsimd.dma_start(
                out=t[:],
                in_=wpT_d[hp * 128:(hp + 1) * 128, ec * 512:(ec + 1) * 512])
            wps.append(t)

        # ---------- filler-step pump ----------
        queue = deque()      # items: [key, deque(steps)]
        done_keys = set()

        def push_group(key, steps):
            queue.append([key, deque(steps)])

        def pump(n):
            while n > 0 and queue:
                key, steps = queue[0]
                steps.popleft()()
                if not steps:
                    done_keys.add(key)
                    queue.popleft()
                n -= 1

        def flush_through(key):
            if key in done_keys:
                return
            while queue:
                k2, steps = queue[0]
                while steps:
                    steps.popleft()()
                done_keys.add(k2)
                queue.popleft()
                if k2 == key:
                    return

        def v_group(tt):
            box = {}

            def step(k):
                def run():
                    if k == 0:
                        box["p"] = ps.tile([128, 512], f32, tag="acc", bufs=2, name="accg")
                    nc.tensor.matmul(box["p"][:], xts[k][:, tt * 128:(tt + 1) * 128],
                                     wvs[k][:], start=(k == 0), stop=(k == NKT - 1))
                    if k == NKT - 1:
                        pvm = box["p"]
                        v3 = vts[tt].rearrange("p (h c) -> p h c", h=HL)
                        # single-op scale-by-gv evacuation: stride-0 broadcast
                        # of the per-(j,h) gv factor along the dh axis
                        gb = gv_sb[:, tt * HL:(tt + 1) * HL].rearrange(
                            "p (h c) -> p h c", c=1).broadcast_to([128, HL, 64])
                        nc.vector.tensor_mul(
                            v3[:, :, 0:64],
                            pvm.rearrange("p (h c) -> p h c", h=HL), gb)
                        nc.vector.tensor_copy(
                            v3[:, :, 64:65],
                            gv_sb[:, tt * HL:(tt + 1) * HL].rearrange(
                                "p (h c) -> p h c", c=1))
                return run
            return [step(k) for k in range(NKT)]

        def qk_group(hp, which, tck):
            wt = wqk[hp][which]
            dest = qts[hp] if which == "q" else kts[hp]
            box = {}

            def step(k):
                def run():
                    if k == 0:
                        box["p"] = ps.tile([128, 512], f32, tag="acc", bufs=2, name="accg")
                    nc.tensor.matmul(box["p"][:], wt[:, k * 128:(k + 1) * 128],
                                     xts[k][:, tck * 512:(tck + 1) * 512],
                                     start=(k == 0), stop=(k == NKT - 1))
                    if k == NKT - 1:
                        nc.vector.tensor_copy(
                            dest[:, tck * 512:(tck + 1) * 512], box["p"][:])
                return run
            return [step(k) for k in range(NKT)]

        def proj_group(tt, ec):
            box = {}

            def step(hp):
                def run():
                    if hp == 0:
                        box["p"] = ps.tile([128, 512], f32, tag="acc", bufs=2, name="accg")
                    nc.tensor.matmul(box["p"][:], oTs[hp][:, tt * 128:(tt + 1) * 128],
                                     wps[hp * 2 + ec][:], start=(hp == 0),
                                     stop=(hp == 3))
                    if hp == 3:
                        ob = sb_l.tile([128, 512], f32, tag="ob", bufs=2)
                        nc.vector.tensor_copy(ob[:], box["p"][:])
                        nc.sync.dma_start(
                            out=out_d[tt * 128:(tt + 1) * 128,
                                      ec * 512:(ec + 1) * 512], in_=ob[:])
                return run
            return [step(hp) for hp in range(4)]

        # queue assembly: segment for chunk c holds work due at chunk c.
        # V(4c),V(4c+1) land before hp0's QK (consumed from jt=2 of (c,0));
        # V(4c+2),V(4c+3) before hp1's.  Chunk 0 leads with hp0's QK so
        # attention starts as soon as wqk0 lands (~7us); V tiles stream behind.
        for c in range(4):
            if c == 0:
                push_group(("qk", "k", 0, 0), qk_group(0, "k", 0))
                push_group(("qk", "q", 0, 0), qk_group(0, "q", 0))
                for tt in range(4):
                    push_group(("V", tt), v_group(tt))
                for hp in range(1, 4):
                    push_group(("qk", "k", hp, 0), qk_group(hp, "k", 0))
                    push_group(("qk", "q", hp, 0), qk_group(hp, "q", 0))
            else:
                for hp in range(4):
                    if hp < 2:
                        push_group(("V", 4 * c + 2 * hp), v_group(4 * c + 2 * hp))
                        push_group(("V", 4 * c + 2 * hp + 1),
                                   v_group(4 * c + 2 * hp + 1))
                    push_group(("qk", "k", hp, c), qk_group(hp, "k", c))
                    push_group(("qk", "q", hp, c), qk_group(hp, "q", c))

        # ---------- normalization (no PE, no PSUM) ----------
        def emit_norm(c, hp, pv0, pv1):
            cs = slice(c * 512, (c + 1) * 512)
            for par, pvx in ((0, pv0), (1, pv1)):
                # copy pv to SBUF first (CAST rounds to f32r) -- this single
                # DVE op releases the PSUM pair for the next head-pair; the
                # rest of the chain (broadcast matmul, base-partition-0
                # approximate reciprocal, muls) reads the copy.
                pvc = sb_l.tile([65, 512], f32r, tag="pvc", bufs=2, name="pvc")
                nc.vector.tensor_copy(pvc[:], pvx[0:65, :])
                bp = ps.tile([64, 512], f32, tag="acc", bufs=2, name="bp")
                nc.tensor.matmul(bp[0:64, :], ones_t[64:65, 0:64],
                                 pvc[64:65, :], start=True, stop=True)
                bb = sb_l.tile([64, 512], f32, tag="bb", bufs=2)
                nc.vector.reciprocal_approx_fast(out=bb[:, :], in_=bp[0:64, :])
                if par == 0:
                    nc.vector.tensor_mul(oTs[hp][0:64, cs], pvc[0:64, :], bb[:])
                else:
                    tm = sb_l.tile([64, 512], bf16, tag="tm", bufs=2)
                    nc.vector.tensor_mul(tm[:], pvc[0:64, :], bb[:])
                    nc.scalar.dma_start(out=oTs[hp][64:128, cs], in_=tm[:])
            if hp == 3:
                for tt in range(4 * c, 4 * c + 4):
                    for ec in range(2):
                        push_group(("proj", tt, ec), proj_group(tt, ec))

        # ---------- attention ----------
        pending = deque()   # (c, hp, jt, njt, p01, cw, pv-box)
        pvbox = {}

        def emit_pv(item):
            c, hp, jt, njt, p01, cw = item
            # the PV matmul reads vts[jt]: its V group must be emitted first
            flush_through(("V", jt))
            if jt == 0:
                pvbox[(c, hp)] = (ps.tile([128, 512], f32, tag="pv", bufs=2, name="pv0"),
                                  ps.tile([128, 512], f32, tag="pv", bufs=2, name="pv1"))
            pv0, pv1 = pvbox[(c, hp)]
            h0, h1 = 0, 1
            nc.tensor.matmul(pv0[0:65, 512 - cw:512],
                             vts[jt][:, (2 * hp) * 65:(2 * hp) * 65 + 65],
                             p01[:, 0:cw], start=(jt == 0), stop=(jt == njt - 1))
            nc.tensor.matmul(pv1[0:65, 512 - cw:512],
                             vts[jt][:, (2 * hp + 1) * 65:(2 * hp + 1) * 65 + 65],
                             p01[:, 512:512 + cw],
                             start=(jt == 0), stop=(jt == njt - 1))
            if jt == njt - 1:
                emit_norm(c, hp, pv0, pv1)

        for c in range(4):
            njt = 4 * c + 4
            for hp in range(4):
                flush_through(("qk", "q", hp, c))
                qt, kt = qts[hp], kts[hp]
                for jt in range(njt):
                    r = jt - 4 * c
                    # bf16 matmuls run 1 cyc/row at any width: use exact
                    # diagonal windows (the mask region is always 128 wide)
                    cw = 512 - 128 * r if r > 0 else 512
                    mw = 128
                    ioff = c * 512 + (512 - cw)
                    s01 = ps.tile([128, 1024], f32, tag="s", bufs=2)
                    nc.tensor.matmul(s01[:, 0:cw], kt[0:64, jt * 128:(jt + 1) * 128],
                                     qt[0:64, ioff:ioff + cw], start=True, stop=True)
                    nc.tensor.matmul(s01[:, 512:512 + cw],
                                     kt[64:128, jt * 128:(jt + 1) * 128],
                                     qt[64:128, ioff:ioff + cw], start=True, stop=True)
                    p01 = sb_l.tile([128, 1024], bf16, tag="pt", bufs=4)
                    s3 = s01.rearrange("p (h m) -> p h m", h=2)
                    p3 = p01.rearrange("p (h m) -> p h m", h=2)
                    nc.scalar.activation(p3[:, :, 0:cw], s3[:, :, 0:cw], Exp)
                    if r >= 0:
                        # zero the j > i region at the head of the window:
                        # keep where (i - j) = (m - (mw - 128)) - pj >= 0
                        for off in (0, 512):
                            nc.gpsimd.affine_select(
                                out=p01[:, off:off + mw], in_=p01[:, off:off + mw],
                                compare_op=mybir.AluOpType.is_ge, fill=0.0,
                                base=-(mw - 128), pattern=[[1, mw]],
                                channel_multiplier=-1)
                    pending.append((c, hp, jt, njt, p01, cw))
                    if len(pending) > LAG:
                        emit_pv(pending.popleft())
                    pump(PUMP)

        while pending:
            emit_pv(pending.popleft())
            pump(PUMP)
        pump(10 ** 9)

    nc.finalize()
    return nc


def _get_nc():
    global _NC_CACHE
    if _NC_CACHE is None:
        _NC_CACHE = _build_nc()
    return _NC_CACHE


def _slopes():
    start = 2.0 ** (-(2.0 ** (-(math.log2(H) - 3.0))))
    return np.array([start * start ** i for i in range(H)], dtype=np.float64)


def _host_prep(x, Wq, Aq, Bq, Wk, Ak, Bk, Wv, Av, Bv, Wp):
    import ml_dtypes
    f8 = np.float64
    weff = {}
    for nm, W, A, B in (("q", Wq, Aq, Bq), ("k", Wk, Ak, Bk), ("v", Wv, Av, Bv)):
        weff[nm] = (W.astype(f8) + 2.0 * (A.astype(f8) @ B.astype(f8)))
    weff["q"] = weff["q"] / math.sqrt(DH)          # fold 1/sqrt(dh) into q weights
    slopes = _slopes()

    jj = np.arange(T, dtype=np.float64).reshape(16, 128).T   # [pj, tt] -> j

    in_maps = []
    for b in range(4):
        xT = np.ascontiguousarray(x[b].T)
        for hg in range(2):
            S = slice(hg * 512, hg * 512 + 512)
            # gv[pj, tt*8 + h] = exp(-(slope_h * j + C)), j = tt*128 + pj
            gv = np.stack([np.exp(-(slopes[hg * 8 + hl] * jj + CB))
                           for hl in range(HL)], axis=2)   # [128, 16, 8]
            gv = gv.reshape(128, 16 * HL).astype(np.float32)
            in_maps.append({
                "xT": xT.astype(ml_dtypes.bfloat16),
                "wqT": np.ascontiguousarray(weff["q"][S].T).astype(ml_dtypes.bfloat16),
                "wkT": np.ascontiguousarray(weff["k"][S].T).astype(ml_dtypes.bfloat16),
                "wvT": np.ascontiguousarray(weff["v"][S].T).astype(ml_dtypes.bfloat16),
                "wpT": np.ascontiguousarray(Wp[:, S].T).astype(ml_dtypes.bfloat16),
                "ebias": gv,
                "onesd": np.ones((128, 128), dtype=np.float32),
            })
    return in_maps


def run(inputs, trace=False):
    nc = _get_nc()
    inputs = {k: np.asarray(v, dtype=np.float32) for k, v in inputs.items()}
    in_maps = _host_prep(**inputs)
    res = run_bass_kernel_spmd(nc, in_maps, list(range(8)), trace=trace)
    outs = [np.asarray(res.results[i]["out"]) for i in range(8)]
    full = np.stack([outs[2 * b] + outs[2 * b + 1] for b in range(4)])
    return full.astype(np.float32), res


def kernel(**inputs):
    full, _ = run(inputs, trace=False)
    return full
